# revision 1
# baseline (speedup 1.0000x reference)
"""ExternalAttention kernel for Trainium2 (8 NeuronCores, batch-parallel).

Math (collapsed from the reference nn.Module):
  q = (poi_data @ wq1 + bq1)[:, 0] @ wq2 + bq2            # [512], shared
  per head h: wkq[:, h] = wk[:, 64h:64h+64] @ q[64h:64h+64] # [512, 8]
  scores = x @ wkq  (+ const per head -- cancels in softmax)
  A = softmax(scores / 8, axis=L)
  xa[h, :] = sum_l A[l, h] * x[l, :]                       # [8, 512]
  V[64h:64h+64] = xa[h] @ wv[:, 64h:64h+64]                # [512]
  row = (V / Z) @ wo + (bv @ wo + bo)                      # [512]
  out[b, l, :] = row_b  for every l.

Sharding: data-parallel over B (8 batch elements = 8 cores); the tiny
shared weights are replicated. Each core streams its x_b once from HBM
through a software-pipelined transpose/score/accumulate loop, then
projects and broadcast-writes the single output row.
"""

import os
import sys

import numpy as np

for _p in ("/opt/trn_rl_repo", "/opt/pypackages"):
    if os.path.isdir(_p) and _p not in sys.path:
        sys.path.append(_p)

B, L, D = 8, 8192, 512
H, DH = 8, 64
P = 128
NCHUNK = L // P  # 64
NJ = D // P  # 4
SCALE = 1.0 / np.sqrt(DH)  # 0.125
N_CORES = 8

_CACHE = {}


def _build_bass():
    import concourse.bass as bass
    import concourse.tile as tile
    from concourse import mybir
    from concourse.bacc import Bacc

    f32 = mybir.dt.float32
    ts = bass.ts

    nc = Bacc(num_swdge_queues=4)
    x_d = nc.dram_tensor("x", [L, D], f32, kind="ExternalInput")
    wkq_d = nc.dram_tensor("wkq", [D, H], f32, kind="ExternalInput")
    wv_d = nc.dram_tensor("wv", [D, D], f32, kind="ExternalInput")
    wo_d = nc.dram_tensor("wo", [D, D], f32, kind="ExternalInput")
    bo2_d = nc.dram_tensor("bo2", [1, D], f32, kind="ExternalInput")
    id_d = nc.dram_tensor("ident", [P, P], f32, kind="ExternalInput")
    m84_d = nc.dram_tensor("m84", [H, NJ], f32, kind="ExternalInput")
    s82_d = nc.dram_tensor("s82", [H, 2], f32, kind="ExternalInput")
    ea2_d = nc.dram_tensor("ea2", [2, P], f32, kind="ExternalInput")
    row_d = nc.dram_tensor("row_scratch", [1, D], f32)
    out_d = nc.dram_tensor("out", [L, D], f32, kind="ExternalOutput")

    with tile.TileContext(nc) as tc:
        with (
            tc.tile_pool(name="consts", bufs=1) as consts,
            tc.tile_pool(name="xin", bufs=16) as xin,
            tc.tile_pool(name="xt", bufs=10) as xtp,
            tc.tile_pool(name="pp", bufs=10) as ppp,
            tc.tile_pool(name="epi", bufs=1) as epi,
        ):
            id128 = consts.tile([P, P], f32)
            nc.scalar.dma_start(id128, id_d[:])
            id1 = consts.tile([1, 1], f32)
            nc.vector.memset(id1, 1.0)
            ones_col = consts.tile([P, 1], f32)
            nc.vector.memset(ones_col, 1.0)

            wkq_sb = consts.tile([P, NJ, H], f32)
            nc.scalar.dma_start(wkq_sb, wkq_d.rearrange("(j p) h -> p j h", p=P))
            wv_sb = consts.tile([P, NJ, D], f32)
            wo_sb = consts.tile([P, NJ, D], f32)
            bo2_sb = consts.tile([1, D], f32)
            m84_sb = consts.tile([H, NJ], f32)
            s82_sb = consts.tile([H, 2], f32)
            ea2_sb = consts.tile([2, P], f32)

            # Per-partition partial softmax denominators, summed over
            # partitions once in the epilogue.
            zacc_sb = epi.tile([P, H], f32)
            nc.vector.memset(zacc_sb, 0.0)

            xa_sb = epi.tile([P, NJ, H], f32)
            z128_sb = epi.tile([P, NJ], f32)

            with tc.tile_pool(name="ps_acc", bufs=1, space="PSUM") as ps_acc:
                # Persistent xa^T accumulators, one PSUM bank per d-slice
                # so each holds exactly one open accumulation group.
                xa_ps = [
                    ps_acc.tile([P, H], f32, name=f"xa{j}", tag=f"xa{j}")
                    for j in range(NJ)
                ]

                with (
                    tc.tile_pool(name="ps_t", bufs=3, space="PSUM") as ps_t,
                    tc.tile_pool(name="ps_s", bufs=1, space="PSUM") as ps_s,
                ):
                    xv = x_d.rearrange("(n p) d -> n p d", p=P)
                    # Software pipeline with a 2-step skew so PE never waits
                    # on the DVE/ACT copy or the exp between its own
                    # instructions: step c = transpose(c), scores(c-1),
                    # accumulate(c-2).
                    xs, xts, ps = {}, {}, {}
                    for c in range(NCHUNK + 2):
                        if c < NCHUNK:
                            x_t = xin.tile([P, D], f32)
                            if c == 0:
                                # split the first load so the pipeline fills
                                # as fast as both queues allow
                                nc.sync.dma_start(x_t[:, 0:256], xv[c][:, 0:256])
                                nc.gpsimd.dma_start(
                                    x_t[:, 256:D], xv[c][:, 256:D]
                                )
                            else:
                                dma_eng = nc.sync if c % 2 == 0 else nc.gpsimd
                                dma_eng.dma_start(x_t, xv[c])
                            xs[c] = x_t

                            xt_ps = ps_t.tile([P, D], f32)
                            for j in range(NJ):
                                nc.tensor.transpose(
                                    xt_ps[:, ts(j, P)], x_t[:, ts(j, P)], id128
                                )
                            xt_sb = xtp.tile([P, D], f32)
                            nc.vector.tensor_copy(
                                xt_sb[:, 0:344], xt_ps[:, 0:344]
                            )
                            nc.scalar.copy(xt_sb[:, 344:D], xt_ps[:, 344:D])
                            xts[c] = xt_sb

                        if 1 <= c <= NCHUNK:
                            cc = c - 1
                            s_ps = ps_s.tile([P, H], f32)
                            for j in range(NJ):
                                nc.tensor.matmul(
                                    s_ps,
                                    xts[cc][:, ts(j, P)],
                                    wkq_sb[:, j, :],
                                    start=(j == 0),
                                    stop=(j == NJ - 1),
                                )
                            p_sb = ppp.tile([P, H], f32)
                            nc.scalar.activation(
                                p_sb,
                                s_ps,
                                mybir.ActivationFunctionType.Exp,
                                scale=SCALE,
                            )
                            ps[cc] = p_sb

                        if c >= 2:
                            cc = c - 2
                            nc.gpsimd.tensor_add(zacc_sb, zacc_sb, ps[cc])
                            for j in range(NJ):
                                nc.tensor.matmul(
                                    xa_ps[j],
                                    xs[cc][:, ts(j, P)],
                                    ps[cc],
                                    start=(cc == 0),
                                    stop=(cc == NCHUNK - 1),
                                )
                            del xs[cc], ps[cc]
                            if cc - 1 in xts:
                                del xts[cc - 1]

                # epilogue-only constants -- load after the stream
                nc.sync.dma_start(wv_sb, wv_d.rearrange("(j p) n -> p j n", p=P))
                nc.sync.dma_start(wo_sb, wo_d.rearrange("(j p) n -> p j n", p=P))
                nc.gpsimd.dma_start(bo2_sb, bo2_d[:])
                nc.gpsimd.dma_start(m84_sb, m84_d[:])
                nc.gpsimd.dma_start(s82_sb, s82_d[:])
                nc.gpsimd.dma_start(ea2_sb, ea2_d[:])

                # drain accumulators; build the [128, 4] normalization grid
                # z128[p, j] = 1 / Z[2j + p//64] from Z via two 0/1 matmuls
                with tc.tile_pool(name="pe0", bufs=1, space="PSUM") as pe0:
                    for j in range(NJ):
                        nc.vector.tensor_copy(xa_sb[:, j, :], xa_ps[j])

                    z_ps = pe0.tile([1, H], f32, tag="t0")
                    nc.tensor.matmul(z_ps, ones_col, zacc_sb)
                    zr_sb = epi.tile([1, H], f32)
                    nc.vector.reciprocal(zr_sb, z_ps)

                    zrt_ps = pe0.tile([H, 1], f32, tag="t0")
                    nc.tensor.transpose(zrt_ps, zr_sb, id1)
                    zrt_sb = epi.tile([H, 1], f32)
                    nc.vector.tensor_copy(zrt_sb, zrt_ps)

                    b_sb = epi.tile([H, NJ], f32)
                    nc.vector.tensor_scalar_mul(b_sb, m84_sb, zrt_sb)
                    r2_ps = pe0.tile([2, NJ], f32, tag="t0")
                    nc.tensor.matmul(r2_ps, s82_sb, b_sb)
                    r2_sb = epi.tile([2, NJ], f32)
                    nc.vector.tensor_copy(r2_sb, r2_ps)
                    z128_ps = pe0.tile([P, NJ], f32, tag="t0")
                    nc.tensor.matmul(z128_ps, ea2_sb, r2_sb)
                    nc.vector.tensor_copy(z128_sb, z128_ps)

            # ---- project V directly in transposed [128, .] layout ----
            with tc.tile_pool(name="pe1", bufs=1, space="PSUM") as pe1:
                vt_sb = epi.tile([P, NJ], f32)
                for j in range(NJ):
                    vtj = pe1.tile([P, 2], f32, name=f"vt{j}", tag=f"vt{j}")
                    for k in range(NJ):
                        nc.tensor.matmul(
                            vtj,
                            wv_sb[:, k, ts(j, P)],
                            xa_sb[:, k, 2 * j : 2 * j + 2],
                            start=(k == 0),
                            stop=(k == NJ - 1),
                        )
                    # h = 2j + p//64: lower half takes column 0, upper column 1
                    nc.vector.tensor_copy(vt_sb[0:64, j : j + 1], vtj[0:64, 0:1])
                    nc.vector.tensor_copy(
                        vt_sb[64:P, j : j + 1], vtj[64:P, 1:2]
                    )

                vtn_sb = epi.tile([P, NJ], f32)
                nc.vector.tensor_mul(vtn_sb, vt_sb, z128_sb)

                row_ps = pe1.tile([1, D], f32, tag="row")
                for j in range(NJ):
                    nc.tensor.matmul(
                        row_ps,
                        vtn_sb[:, j : j + 1],
                        wo_sb[:, j, :],
                        start=(j == 0),
                        stop=(j == NJ - 1),
                    )
                row_sb = epi.tile([1, D], f32)
                nc.vector.tensor_add(row_sb, row_ps, bo2_sb)

                # broadcast write: bounce the row through DRAM, fill a
                # [128, 4, 512] SBUF tile (4 row copies per partition) via a
                # DRAM-side stride-0 broadcast, then write the output as 16
                # one-MB DMAs whose per-partition runs are 8 KB contiguous.
                r_sb = epi.tile([P, D], f32)
                nc.gpsimd.partition_broadcast(r_sb, row_sb)
                ov = out_d.rearrange("(n p) d -> n p d", p=P)
                w_engines = [nc.sync, nc.gpsimd, nc.scalar]
                for c in range(NCHUNK):
                    w_engines[c % len(w_engines)].dma_start(ov[c], r_sb)

    if not nc.is_finalized():
        nc.finalize()
    return nc


def _get_nc():
    if "nc" not in _CACHE:
        _CACHE["nc"] = _build_bass()
    return _CACHE["nc"]


def _host_prep(inputs):
    poi = np.asarray(inputs["poi_data"], np.float32)
    wq1 = np.asarray(inputs["wq1"], np.float32)
    bq1 = np.asarray(inputs["bq1"], np.float32)
    wq2 = np.asarray(inputs["wq2"], np.float32)
    bq2 = np.asarray(inputs["bq2"], np.float32)
    wk = np.asarray(inputs["wk"], np.float32)

    q1 = (poi @ wq1 + bq1)[:, 0]  # [1683]
    q = q1 @ wq2 + bq2  # [512]
    qh = q.reshape(H, DH)
    wkq = np.stack(
        [wk[:, h * DH : (h + 1) * DH] @ qh[h] for h in range(H)], axis=1
    )  # [512, 8]
    return wkq.astype(np.float32)


def _make_in_maps(inputs):
    x = np.ascontiguousarray(np.asarray(inputs["x"], np.float32))
    wv = np.ascontiguousarray(np.asarray(inputs["wv"], np.float32))
    wo = np.ascontiguousarray(np.asarray(inputs["wo"], np.float32))
    bv = np.asarray(inputs["bv"], np.float32).reshape(D)
    bo = np.asarray(inputs["bo"], np.float32).reshape(D)
    wkq = _host_prep(inputs)

    bo2 = (bv @ wo + bo).reshape(1, D).astype(np.float32)
    hh = np.arange(H)
    m84 = (hh[:, None] // 2 == np.arange(NJ)[None, :]).astype(np.float32)
    s82 = (hh[:, None] % 2 == np.arange(2)[None, :]).astype(np.float32)
    ea2 = (np.arange(2)[:, None] == (np.arange(P)[None, :] // 64)).astype(
        np.float32
    )
    ident = np.eye(P, dtype=np.float32)

    return [
        {
            "x": np.ascontiguousarray(x[b]),
            "wkq": wkq,
            "wv": wv,
            "wo": wo,
            "bo2": bo2,
            "ident": ident,
            "m84": m84,
            "s82": s82,
            "ea2": ea2,
        }
        for b in range(N_CORES)
    ]


def kernel(**inputs) -> np.ndarray:
    from concourse.bass_utils import run_bass_kernel_spmd

    nc = _get_nc()
    in_maps = _make_in_maps(inputs)
    res = run_bass_kernel_spmd(nc, in_maps, list(range(N_CORES)))
    out = np.stack([res.results[b]["out"] for b in range(N_CORES)], axis=0)
    return out.astype(np.float32)



# revision 11
# speedup vs baseline: 1.5210x; 1.5210x over previous
"""ExternalAttention kernel for Trainium2 (8 NeuronCores, batch-parallel).

Math (collapsed from the reference nn.Module):
  q = (poi_data @ wq1 + bq1)[:, 0] @ wq2 + bq2            # [512], shared
  per head h: wkq[:, h] = wk[:, 64h:64h+64] @ q[64h:64h+64] # [512, 8]
  scores = x @ wkq  (+ const per head -- cancels in softmax)
  A = softmax(scores / 8, axis=L)
  xa[h, :] = sum_l A[l, h] * x[l, :]                       # [8, 512]
  V[64h:64h+64] = xa[h] @ wv[:, 64h:64h+64]                # [512]
  row = (V / Z) @ wo + (bv @ wo + bo)                      # [512]
  out[b, l, :] = row_b  for every l.

Sharding: data-parallel over B (8 batch elements = 8 cores); the tiny
shared weights are replicated.

Stream design: 64 chunks of 128 rows, in 8 groups of 8.  Per group, 6
chunks are cast-loaded to bf16 on the gpsimd queue, 1 f32 chunk each on
the sync and scalar queues, so all three DMA-capable engines share the
read traffic.  Per chunk: PE transposes the 4 [128,128] d-slices
(bf16 1 cyc/row, f32 via a float32r bitcast), DVE/ACT copy the
transposed slab PSUM->SBUF casting to bf16, 4 tiny bf16 matmuls
accumulate scores into a per-group [128, 64] PSUM tile.  One exp per
group (scalar engine) yields the 8 chunks' softmax numerators, which
feed per-chunk accumulation matmuls into a persistent [128, 4, 8]
xa^T PSUM tile.  The epilogue projects the row and writes the output
with a single stride-0-source broadcast DMA.
"""

import os
import sys

import numpy as np

for _p in ("/opt/trn_rl_repo", "/opt/pypackages"):
    if os.path.isdir(_p) and _p not in sys.path:
        sys.path.append(_p)

B, L, D = 8, 8192, 512
H, DH = 8, 64
P = 128
NCHUNK = L // P  # 64
NJ = D // P  # 4
G = 8  # chunks per score/exp group
NG = NCHUNK // G  # 8 groups
NBF = 6  # bf16 (gpsimd cast) chunks per group; slots 0..5
SCALE = 1.0 / np.sqrt(DH)  # 0.125
N_CORES = 8

_CACHE = {}


def _build_bass():
    import concourse.bass as bass
    import concourse.tile as tile
    from concourse import mybir
    from concourse.bacc import Bacc

    f32 = mybir.dt.float32
    f32r = mybir.dt.float32r
    bf16 = mybir.dt.bfloat16
    ts = bass.ts

    nc = Bacc(num_swdge_queues=4)
    x_d = nc.dram_tensor("x", [L, D], f32, kind="ExternalInput")
    wkqb_d = nc.dram_tensor("wkqb", [D, H], bf16, kind="ExternalInput")
    wv_d = nc.dram_tensor("wv", [D, D], f32, kind="ExternalInput")
    wob_d = nc.dram_tensor("wob", [D, D], bf16, kind="ExternalInput")
    bo2_d = nc.dram_tensor("bo2", [1, D], f32, kind="ExternalInput")
    idb_d = nc.dram_tensor("identb", [P, P], bf16, kind="ExternalInput")
    idf_d = nc.dram_tensor("ident", [P, P], f32, kind="ExternalInput")
    m84_d = nc.dram_tensor("m84", [H, NJ], f32, kind="ExternalInput")
    s82_d = nc.dram_tensor("s82", [H, 2], f32, kind="ExternalInput")
    ea2_d = nc.dram_tensor("ea2", [2, P], f32, kind="ExternalInput")
    row_d = nc.dram_tensor("row_scratch", [1, D], f32)
    out_d = nc.dram_tensor("out", [L, D], f32, kind="ExternalOutput")

    with tile.TileContext(nc) as tc:
        with (
            tc.tile_pool(name="consts", bufs=1) as consts,
            tc.tile_pool(name="xb", bufs=2) as xbp,
            tc.tile_pool(name="xs", bufs=2) as xsp,
            tc.tile_pool(name="xa_", bufs=2) as xap,
            tc.tile_pool(name="xt", bufs=4) as xtp,
            tc.tile_pool(name="pg", bufs=2) as pgp,
            tc.tile_pool(name="pg8", bufs=4) as pg8p,
            tc.tile_pool(name="epi", bufs=1) as epi,
        ):
            idb = consts.tile([P, P], bf16)
            nc.sync.dma_start(idb, idb_d[:])
            idf = consts.tile([P, P], f32)
            nc.sync.dma_start(idf, idf_d[:])
            id1 = consts.tile([1, 1], f32)
            nc.vector.memset(id1, 1.0)
            ones_col = consts.tile([P, 1], f32)
            nc.vector.memset(ones_col, 1.0)

            wkq_sb = consts.tile([P, NJ, H], bf16)
            nc.sync.dma_start(wkq_sb, wkqb_d.rearrange("(j p) h -> p j h", p=P))
            wv_sb = consts.tile([P, NJ, D], f32)
            wob_sb = consts.tile([P, NJ, D], bf16)
            bo2_sb = consts.tile([1, D], f32)
            m84_sb = consts.tile([H, NJ], f32)
            s82_sb = consts.tile([H, 2], f32)
            ea2_sb = consts.tile([2, P], f32)
            nc.gpsimd.dma_start(bo2_sb, bo2_d[:])
            nc.gpsimd.dma_start(m84_sb, m84_d[:])
            nc.gpsimd.dma_start(s82_sb, s82_d[:])
            nc.gpsimd.dma_start(ea2_sb, ea2_d[:])

            # Per-partition softmax-denominator partials, summed over
            # partitions and chunk-slots in the epilogue.
            zacc_sb = epi.tile([P, G, H], f32)
            nc.vector.memset(zacc_sb, 0.0)

            xa_sb = epi.tile([P, NJ, H], f32)
            z128_sb = epi.tile([P, NJ], f32)

            xv = x_d.rearrange("(n p) d -> n p d", p=P)
            xpm = x_d.rearrange("(n p) d -> p n d", p=P)

            with tc.tile_pool(name="ps_acc", bufs=1, space="PSUM") as ps_acc:
                # Persistent xa^T accumulator: 4 column regions (one per
                # d-slice), each an open accumulation group over all 64
                # chunks.
                xa_ps = ps_acc.tile([P, NJ, H], f32, name="xa", tag="xa")

                with (
                    tc.tile_pool(name="ps_tb", bufs=3, space="PSUM") as ps_tb,
                    tc.tile_pool(name="ps_tf", bufs=2, space="PSUM") as ps_tf,
                    tc.tile_pool(name="ps_s", bufs=2, space="PSUM") as ps_s,
                ):
                    for g in range(NG):
                        # ---- group loads: 6 bf16 chunks on gpsimd, one
                        # f32 chunk each on sync and scalar ----
                        xg_b = xbp.tile([P, NBF, D], bf16)
                        nc.gpsimd.dma_start(
                            xg_b, xpm[:, G * g : G * g + NBF, :]
                        )
                        x_s = xsp.tile([P, D], f32)
                        nc.sync.dma_start(x_s, xv[G * g + 6])
                        x_a = xap.tile([P, D], f32)
                        nc.scalar.dma_start(x_a, xv[G * g + 7])
                        if g == 0:
                            # overlap the fat epilogue weights with the
                            # stream (sync queue has slack)
                            nc.sync.dma_start(
                                wv_sb, wv_d.rearrange("(j p) n -> p j n", p=P)
                            )
                            nc.sync.dma_start(
                                wob_sb, wob_d.rearrange("(j p) n -> p j n", p=P)
                            )

                        s8_ps = ps_s.tile([P, G, H], f32)
                        xts = {}
                        for c in range(G):
                            # ---- transpose chunk c and copy to SBUF bf16 --
                            xt_sb = xtp.tile([P, D], bf16)
                            if c < NBF:
                                src = xg_b[:, c, :]
                                xt_ps = ps_tb.tile([P, D], bf16)
                                for j in range(NJ):
                                    nc.tensor.transpose(
                                        xt_ps[:, ts(j, P)], src[:, ts(j, P)], idb
                                    )
                                nc.vector.tensor_copy(xt_sb, xt_ps)
                            else:
                                src = x_s if c == 6 else x_a
                                xt_ps = ps_tf.tile([P, D], f32)
                                for j in range(NJ):
                                    nc.tensor.transpose(
                                        xt_ps[:, ts(j, P)], src[:, ts(j, P)], idf
                                    )
                                nc.scalar.copy(xt_sb, xt_ps)
                            xts[c] = xt_sb

                            # ---- scores for chunk c into group PSUM ----
                            for j in range(NJ):
                                nc.tensor.matmul(
                                    s8_ps[:, c, :],
                                    xts[c][:, ts(j, P)],
                                    wkq_sb[:, j, :],
                                    start=(j == 0),
                                    stop=(j == NJ - 1),
                                )
                            del xts[c]

                        # ---- group softmax numerators ----
                        pg_b = pgp.tile([P, G, H], bf16)
                        nc.scalar.activation(
                            pg_b,
                            s8_ps,
                            mybir.ActivationFunctionType.Exp,
                            scale=SCALE,
                        )
                        nc.gpsimd.tensor_add(zacc_sb, zacc_sb, pg_b)
                        # f32 copies of the two f32 chunks' weights (the
                        # accum matmul needs dtype-matched operands)
                        pf_s = pg8p.tile([P, H], f32)
                        nc.gpsimd.tensor_copy(pf_s, pg_b[:, 6, :])
                        pf_a = pg8p.tile([P, H], f32)
                        nc.gpsimd.tensor_copy(pf_a, pg_b[:, 7, :])

                        # ---- accumulate xa^T for the group's chunks ----
                        # One umbrella accumulation group for the whole
                        # [P, NJ, H] tile: the very first matmul starts
                        # (clearing the bank's accumulate bits), each
                        # span's own first write then overwrites (bit
                        # unset) and later writes accumulate (bit set).
                        for c in range(G):
                            if c < NBF:
                                src, rhs = xg_b[:, c, :], pg_b[:, c, :]
                            else:
                                src = x_s if c == 6 else x_a
                                rhs = pf_s if c == 6 else pf_a
                            for j in range(NJ):
                                nc.tensor.matmul(
                                    xa_ps[:, j, :],
                                    src[:, ts(j, P)],
                                    rhs,
                                    start=(g == 0 and c == 0 and j == 0),
                                    stop=(
                                        g == NG - 1
                                        and c == G - 1
                                        and j == NJ - 1
                                    ),
                                    skip_group_check=True,
                                )

                # ---- epilogue: drain accumulators; build the [128, 4]
                # normalization grid z128[p, j] = 1 / Z[2j + p//64] ----
                with tc.tile_pool(name="pe0", bufs=1, space="PSUM") as pe0:
                    nc.vector.tensor_copy(xa_sb, xa_ps)

                    zg_ps = pe0.tile([1, G, H], f32, tag="t0")
                    nc.tensor.matmul(zg_ps, ones_col, zacc_sb)
                    zg_sb = epi.tile([1, G, H], f32)
                    nc.vector.tensor_copy(zg_sb, zg_ps)
                    # fold the 8 chunk-slots: Z[h] = sum_c zg[c, h]
                    z_sb = epi.tile([1, H], f32)
                    nc.vector.tensor_copy(z_sb, zg_sb[:, 0, :])
                    for c in range(1, G):
                        nc.vector.tensor_add(z_sb, z_sb, zg_sb[:, c, :])
                    zr_sb = epi.tile([1, H], f32)
                    nc.vector.reciprocal(zr_sb, z_sb)

                    zrt_ps = pe0.tile([H, 1], f32, tag="t0")
                    nc.tensor.transpose(zrt_ps, zr_sb, id1)
                    zrt_sb = epi.tile([H, 1], f32)
                    nc.vector.tensor_copy(zrt_sb, zrt_ps)

                    b_sb = epi.tile([H, NJ], f32)
                    nc.vector.tensor_scalar_mul(b_sb, m84_sb, zrt_sb)
                    r2_ps = pe0.tile([2, NJ], f32, tag="t0")
                    nc.tensor.matmul(r2_ps, s82_sb, b_sb)
                    r2_sb = epi.tile([2, NJ], f32)
                    nc.vector.tensor_copy(r2_sb, r2_ps)
                    z128_ps = pe0.tile([P, NJ], f32, tag="t0")
                    nc.tensor.matmul(z128_ps, ea2_sb, r2_sb)
                    nc.vector.tensor_copy(z128_sb, z128_ps)

            # ---- project V directly in transposed [128, .] layout ----
            with tc.tile_pool(name="pe1", bufs=1, space="PSUM") as pe1:
                vt_sb = epi.tile([P, NJ], f32)
                for j in range(NJ):
                    vtj = pe1.tile([P, 2], f32, name=f"vt{j}", tag=f"vt{j}")
                    for k in range(NJ):
                        nc.tensor.matmul(
                            vtj,
                            wv_sb[:, k, ts(j, P)],
                            xa_sb[:, k, 2 * j : 2 * j + 2],
                            start=(k == 0),
                            stop=(k == NJ - 1),
                        )
                    # h = 2j + p//64: lower half takes column 0, upper column 1
                    nc.vector.tensor_copy(vt_sb[0:64, j : j + 1], vtj[0:64, 0:1])
                    nc.vector.tensor_copy(
                        vt_sb[64:P, j : j + 1], vtj[64:P, 1:2]
                    )

                vtn_sb = epi.tile([P, NJ], f32)
                nc.vector.tensor_mul(vtn_sb, vt_sb, z128_sb)
                vtn_b = epi.tile([P, NJ], bf16)
                nc.vector.tensor_copy(vtn_b, vtn_sb)

                row_ps = pe1.tile([1, D], f32, tag="row")
                for j in range(NJ):
                    nc.tensor.matmul(
                        row_ps,
                        vtn_b[:, j : j + 1],
                        wob_sb[:, j, :],
                        start=(j == 0),
                        stop=(j == NJ - 1),
                    )
                row_sb = epi.tile([1, D], f32)
                nc.vector.tensor_add(row_sb, row_ps, bo2_sb)

                # broadcast write: land the row in DRAM once, then emit a
                # single stride-0-source DMA replicating it across all L
                # output rows.
                nc.sync.dma_start(row_d[:], row_sb)
                src = row_d[:].squeeze(0).partition_broadcast(L)
                nc.sync.dma_start(out_d[:], src)

    if not nc.is_finalized():
        nc.finalize()
    return nc


def _get_nc():
    if "nc" not in _CACHE:
        _CACHE["nc"] = _build_bass()
    return _CACHE["nc"]


def _host_prep(inputs):
    poi = np.asarray(inputs["poi_data"], np.float32)
    wq1 = np.asarray(inputs["wq1"], np.float32)
    bq1 = np.asarray(inputs["bq1"], np.float32)
    wq2 = np.asarray(inputs["wq2"], np.float32)
    bq2 = np.asarray(inputs["bq2"], np.float32)
    wk = np.asarray(inputs["wk"], np.float32)

    q1 = (poi @ wq1 + bq1)[:, 0]  # [1683]
    q = q1 @ wq2 + bq2  # [512]
    qh = q.reshape(H, DH)
    wkq = np.stack(
        [wk[:, h * DH : (h + 1) * DH] @ qh[h] for h in range(H)], axis=1
    )  # [512, 8]
    return wkq.astype(np.float32)


def _make_in_maps(inputs):
    import ml_dtypes

    bf16 = ml_dtypes.bfloat16

    x = np.ascontiguousarray(np.asarray(inputs["x"], np.float32))
    wv = np.ascontiguousarray(np.asarray(inputs["wv"], np.float32))
    wo = np.ascontiguousarray(np.asarray(inputs["wo"], np.float32))
    bv = np.asarray(inputs["bv"], np.float32).reshape(D)
    bo = np.asarray(inputs["bo"], np.float32).reshape(D)
    wkq = _host_prep(inputs)

    bo2 = (bv @ wo + bo).reshape(1, D).astype(np.float32)
    hh = np.arange(H)
    m84 = (hh[:, None] // 2 == np.arange(NJ)[None, :]).astype(np.float32)
    s82 = (hh[:, None] % 2 == np.arange(2)[None, :]).astype(np.float32)
    ea2 = (np.arange(2)[:, None] == (np.arange(P)[None, :] // 64)).astype(
        np.float32
    )
    ident = np.eye(P, dtype=np.float32)

    return [
        {
            "x": np.ascontiguousarray(x[b]),
            "wkqb": wkq.astype(bf16),
            "wv": wv,
            "wob": wo.astype(bf16),
            "bo2": bo2,
            "identb": ident.astype(bf16),
            "ident": ident,
            "m84": m84,
            "s82": s82,
            "ea2": ea2,
        }
        for b in range(N_CORES)
    ]


def kernel(**inputs) -> np.ndarray:
    from concourse.bass_utils import run_bass_kernel_spmd

    nc = _get_nc()
    in_maps = _make_in_maps(inputs)
    res = run_bass_kernel_spmd(nc, in_maps, list(range(N_CORES)))
    out = np.stack([res.results[b]["out"] for b in range(N_CORES)], axis=0)
    return out.astype(np.float32)
